# revision 37
# baseline (speedup 1.0000x reference)
"""Trainium2 Bass kernel for nn_MultiHeadMchAttnBlock.

Reference computation (B=4, M=1024, ND=64, ED=8, D=64, H=4):
    Wh   = einsum('bmd,hde->bhme', h, W)            # [B,H,M,D]
    Wh1  = Wh @ a1, Wh2 = Wh @ a2                   # [B,H,M]
    w_e  = einsum('hed,hd->he', W_edge, a3)         # [H,ED]
    ef   = einsum('bkqe,he->bhkq', comp_val, w_e)   # [B,H,M,M]
    e    = leaky_relu(Wh1[...,None] + Wh2[...,None,:] + ef, 0.2)
    e    = where(mask, e, -9e15)
    attn = softmax(e, axis=-1)
    out  = concat_heads(attn @ Wh)                  # [B,M,H*D]

Sharding: data-parallel over the k axis (rows of the attention matrix).
Core c handles k in [c*128, (c+1)*128) for all (b, h).  Each core reads a
disjoint 1/8 slice of comp_val (the dominant 128MB tensor) and of the mask.

Small weight-derived tensors (Wh, Wh1, Wh2, w_e block-diag matrix, the
additive mask bias) are precomputed on host in numpy — they are tiny
compared to comp_val and avoid wasting device passes.

Device pipeline, fully fused per (b, q-block of 128):
  PE  : transpose comp_val tiles (bf16 PSUM via is_transpose)
  DVE : copy T -> SBUF (packed bf16 2x reads)
  PE  : Wh1/Wh2 fold (K=5, start=True full-width first: PSUM has_written
        resets per bank on start=True), ef matmuls (T stationary vs
        block-diag w_e -> [k,(h,q)] layout), mask add (identity matmul)
  ACT : leaky_relu (Prelu alpha=0.2) PSUM -> SBUF, then exp (unnormalized)
  PE  : transpose attn block per head; final matmul accumulates h_prime
        AND the softmax denominator via a ones-column appended to Wh
epilogue per b: reciprocal of the Z column, scale h_prime rows, DMA out.
"""

import sys

sys.path.insert(0, "/opt/trn_rl_repo")

import numpy as np
from contextlib import ExitStack

import concourse.bass as bass
import concourse.bacc as bacc
import concourse.tile as tile
from concourse.tile import add_dep_helper
from concourse import mybir
from concourse.bass_utils import run_bass_kernel_spmd
from concourse.masks import make_identity

BF16 = mybir.dt.bfloat16
F32 = mybir.dt.float32
NP_BF16 = mybir.dt.np(BF16)

B, M, ND, ED, D, H = 4, 1024, 64, 8, 64, 4
ALPHA = 0.2
NCORES = 8
KS = M // NCORES  # 128 k-rows per core
Q = M
NEG_BIG = -1.0e30

_compiled = {}


def build_nc():
    import os

    skip_logits = os.environ.get("K_SKIP_LOGITS") == "1"
    skip_softmax = os.environ.get("K_SKIP_SOFTMAX") == "1"
    bufs = lambda name, dflt: int(os.environ.get(f"K_BUFS_{name}", dflt))
    nc = bacc.Bacc()

    CV = nc.declare_dram_parameter("cv", [B, KS, Q * ED], F32, isOutput=False)
    MA = nc.declare_dram_parameter("maskadd", [B, KS, Q], BF16, isOutput=False)
    WH = nc.declare_dram_parameter("wh", [B, KS, H * 8 * (D + 1)], BF16, isOutput=False)
    R5 = nc.declare_dram_parameter("rhs5", [B, 5, H * Q], BF16, isOutput=False)
    L5 = nc.declare_dram_parameter("lhsT5", [B, 5, KS], BF16, isOutput=False)
    BDT = nc.declare_dram_parameter("bdt", [128, 64], BF16, isOutput=False)
    OUT = nc.declare_dram_parameter("out", [B, KS, H * D], F32, isOutput=True)

    with tile.TileContext(nc) as tc, ExitStack() as ctx:
        const = ctx.enter_context(tc.tile_pool(name="const", bufs=1))
        sb_cv = ctx.enter_context(tc.tile_pool(name="sb_cv", bufs=bufs("sb_cv", 2)))
        sb_T = ctx.enter_context(tc.tile_pool(name="sb_T", bufs=bufs("sb_T", 3)))
        sb_e = ctx.enter_context(tc.tile_pool(name="sb_e", bufs=bufs("sb_e", 2)))
        sb_a = ctx.enter_context(tc.tile_pool(name="sb_a", bufs=bufs("sb_a", 2)))
        sb_at = ctx.enter_context(tc.tile_pool(name="sb_at", bufs=bufs("sb_at", 2)))
        sb_wh = ctx.enter_context(tc.tile_pool(name="sb_wh", bufs=2))
        sb_ma = ctx.enter_context(tc.tile_pool(name="sb_ma", bufs=2))
        sb_r5 = ctx.enter_context(tc.tile_pool(name="sb_r5", bufs=2))
        sb_l5 = ctx.enter_context(tc.tile_pool(name="sb_l5", bufs=2))
        sb_z = ctx.enter_context(tc.tile_pool(name="sb_z", bufs=4))
        sb_out = ctx.enter_context(tc.tile_pool(name="sb_out", bufs=2))
        ps_T = ctx.enter_context(tc.tile_pool(name="ps_T", bufs=bufs("ps_T", 2), space="PSUM"))
        ps_e = ctx.enter_context(tc.tile_pool(name="ps_e", bufs=bufs("ps_e", 3), space="PSUM"))
        ps_at = ctx.enter_context(tc.tile_pool(name="ps_at", bufs=bufs("ps_at", 2), space="PSUM"))
        ps_hp = ctx.enter_context(tc.tile_pool(name="ps_hp", bufs=bufs("ps_hp", 1), space="PSUM"))

        ident = const.tile([128, 128], BF16)
        make_identity(nc, ident)
        bdt_t = const.tile([128, 64], BF16)
        nc.sync.dma_start(out=bdt_t, in_=BDT[:])
        zrow_t = const.tile([1, 128], BF16)
        nc.vector.memset(zrow_t, 0.0)
        zcol_t = const.tile([1, H * (D + 1)], BF16)
        nc.vector.memset(zcol_t, 0.0)

        for b in range(B):
            # ---- loads for batch b ----
            cv_t = sb_cv.tile([128, Q * ED], BF16, tag="cv")
            # finer chunks for b=0 so the PE pipeline starts sooner;
            # coarser afterwards to cut SWDGE descriptor overhead.
            ncv = int(os.environ.get("K_CV_CHUNKS", "8" if b == 0 else "4"))
            w = (Q * ED) // ncv
            for c in range(ncv):
                nc.gpsimd.dma_start(
                    out=cv_t[:, c * w : (c + 1) * w],
                    in_=CV[b][:, c * w : (c + 1) * w],
                )
            ma_t = sb_ma.tile([128, Q], BF16, tag="ma")
            nc.sync.dma_start(out=ma_t, in_=MA[b])
            wh_t = sb_wh.tile([128, H, 8, D + 1], BF16, tag="wh")
            nc.sync.dma_start(out=wh_t, in_=WH[b].rearrange("p (h c d) -> p h c d", h=H, c=8))
            r5_t = sb_r5.tile([5, H, Q], BF16, tag="r5")
            nc.sync.dma_start(out=r5_t, in_=R5[b].rearrange("p (h q) -> p h q", h=H))
            l5_t = sb_l5.tile([5, 128], BF16, tag="l5")
            nc.sync.dma_start(out=l5_t, in_=L5[b])

            e_t = sb_e.tile([128, H, Q], BF16, tag="e")
            a_t = sb_a.tile([128, H, Q], BF16, tag="a")
            if skip_logits:
                nc.vector.memset(e_t[:, :, 0:8], 0.0)
                nc.vector.memset(a_t[:, :, 0:8], 0.0)

            # h_prime accumulator for all heads; zero-init with one
            # full-width start=True matmul so every later matmul can be a
            # plain accumulate (PSUM has_written resets per start=True).
            if not skip_softmax:
                hp_ps = ps_hp.tile([128, H, D + 1], F32, tag="hp")
                hp_init = nc.tensor.matmul(
                    hp_ps.rearrange("p h d -> p (h d)"),
                    lhsT=zrow_t,
                    rhs=zcol_t,
                    start=True,
                    stop=False,
                    skip_group_check=True,
                )

            # ---- logits + attn per q-block of 128 ----
            for qb in range(0 if not skip_logits else 8, 8):
                # transpose cv tiles (bf16 PSUM out via is_transpose) and
                # evacuate with one packed bf16 copy per block.
                T_ps = ps_T.tile([128, 8, 128], BF16, tag="Tps")
                for t in range(8):
                    nc.tensor.transpose(
                        T_ps[:, t, :],
                        cv_t[:, qb * 1024 + t * 128 : qb * 1024 + (t + 1) * 128],
                        ident,
                    )
                T_sb = sb_T.tile([128, 8, 128], BF16, tag="Tsb")
                nc.vector.tensor_copy(out=T_sb, in_=T_ps)

                # PSUM has_written semantics: a start=True matmul resets the
                # whole bank's accumulate state, so the FIRST matmul must be
                # the full-width fold; everything else accumulates after it.
                # Tile treats accumulating matmuls as reorderable, so pin the
                # order with explicit dep edges.
                e_ps = ps_e.tile([128, H, 128], F32, tag="eps")
                fold = nc.tensor.matmul(
                    e_ps[:, :, :],
                    lhsT=l5_t,
                    rhs=r5_t[:, :, qb * 128 : (qb + 1) * 128],
                    start=True,
                    stop=False,
                )
                prev = fold
                for t in range(8):
                    mm = nc.tensor.matmul(
                        e_ps[:, :, t * 16 : (t + 1) * 16],
                        lhsT=T_sb[:, t, :],
                        rhs=bdt_t,
                        start=False,
                        stop=False,
                    )
                    add_dep_helper(mm.ins, prev.ins, sync=False, reason="accum order")
                    prev = mm
                for hh in range(H):
                    mm = nc.tensor.matmul(
                        e_ps[:, hh, :],
                        lhsT=ident,
                        rhs=ma_t[:, qb * 128 : (qb + 1) * 128],
                        start=False,
                        stop=(hh == H - 1),
                    )
                    add_dep_helper(mm.ins, prev.ins, sync=False, reason="accum order")
                    prev = mm
                nc.scalar.activation(
                    e_t[:, :, qb * 128 : (qb + 1) * 128],
                    e_ps,
                    mybir.ActivationFunctionType.Prelu,
                    alpha=ALPHA,
                )
                if skip_softmax:
                    continue

                # unnormalized attention for this block (1/Z is applied to
                # h_prime at the very end), then transpose per head and
                # accumulate the final matmul — all within the block loop so
                # nothing serializes behind the full softmax row.
                nc.scalar.activation(
                    a_t[:, :, qb * 128 : (qb + 1) * 128],
                    e_t[:, :, qb * 128 : (qb + 1) * 128],
                    mybir.ActivationFunctionType.Exp,
                )
                at_ps = ps_at.tile([128, H, 128], BF16, tag="atps")
                for hh in range(H):
                    nc.tensor.transpose(
                        at_ps[:, hh, :],
                        a_t[:, hh, qb * 128 : (qb + 1) * 128],
                        ident,
                    )
                at_sb = sb_at.tile([128, H, 128], BF16, tag="atsb")
                nc.vector.tensor_copy(out=at_sb, in_=at_ps)
                for hh in range(H):
                    mm = nc.tensor.matmul(
                        hp_ps[:, hh, :],
                        lhsT=at_sb[:, hh, :],
                        rhs=wh_t[:, hh, qb, :],
                        start=False,
                        stop=(qb == 7 and hh == H - 1),
                        skip_group_check=True,
                    )
                    # accumulates commute; only the zero-init must precede
                    add_dep_helper(mm.ins, hp_init.ins, sync=False, reason="hp after init")

            # ---- epilogue: row sums, normalize, store ----
            if skip_softmax:
                continue
            out_t = sb_out.tile([128, H, D], F32, tag="out")
            for hh in range(H):
                r_t = sb_z.tile([128, 1], F32, tag=f"r{hh}")
                nc.vector.reciprocal(out=r_t, in_=hp_ps[:, hh, D : D + 1])
                nc.vector.tensor_scalar_mul(out_t[:, hh, :], hp_ps[:, hh, 0:D], r_t)
            nc.sync.dma_start(out=OUT[b], in_=out_t.rearrange("p h d -> p (h d)"))

    nc.finalize()
    return nc


def _host_prep(h, mch_mask, comp_val, W, W_edge, a):
    """Compute small derived tensors on host; build per-core input maps."""
    d = W.shape[-1]
    a1, a2, a3 = a[:, :d], a[:, d : 2 * d], a[:, 2 * d :]

    # [H, ND] fused W @ a1 / W @ a2
    wa1 = np.einsum("hde,he->hd", W, a1)
    wa2 = np.einsum("hde,he->hd", W, a2)
    Wh1 = np.einsum("bmd,hd->bhm", h, wa1)  # [B, H, M]
    Wh2 = np.einsum("bmd,hd->bhm", h, wa2)  # [B, H, M]
    Wh = np.einsum("bmd,hde->bhme", h, W)  # [B, H, M, D]
    w_e = np.einsum("hed,hd->he", W_edge, a3)  # [H, ED]

    # block-diag-transposed w_e: bdt[qc*8+e, hh*16+qc] = w_e[hh, e]
    bdt = np.zeros((128, 64), np.float32)
    for qc in range(16):
        for hh in range(H):
            bdt[qc * 8 : qc * 8 + 8, hh * 16 + qc] = w_e[hh]
    bdt = bdt.astype(NP_BF16)

    # rhs5[b]: row0 = Wh2[b,h,q] at (h*Q+q); rows 1+h' = head selector
    rhs5 = np.zeros((B, 5, H * Q), np.float32)
    rhs5[:, 0, :] = Wh2.reshape(B, H * Q)
    for hp in range(H):
        rhs5[:, 1 + hp, hp * Q : (hp + 1) * Q] = 1.0
    rhs5 = rhs5.astype(NP_BF16)

    # wh shipped pre-arranged with a trailing ones column (computes the
    # softmax denominator as the final matmul's last output column):
    # wh_dev[b, p, (h, c, d)] = Wh[b, h, c*128+p, d], d=D -> 1.0
    Wh65 = np.concatenate([Wh, np.ones((B, H, M, 1), np.float32)], axis=-1)
    wh_dev = np.ascontiguousarray(
        Wh65.reshape(B, H, 8, 128, D + 1).transpose(0, 3, 1, 2, 4).reshape(B, 128, H * 8 * (D + 1))
    ).astype(NP_BF16)

    maskadd = ((mch_mask.astype(np.float32) - 1.0) * 1.0e30).astype(NP_BF16)

    in_maps = []
    for core in range(NCORES):
        ks = slice(core * KS, (core + 1) * KS)
        lhsT5 = np.zeros((B, 5, KS), np.float32)
        lhsT5[:, 0, :] = 1.0
        lhsT5[:, 1:5, :] = Wh1[:, :, ks]
        in_maps.append(
            {
                "cv": np.ascontiguousarray(comp_val[:, ks]).reshape(B, KS, Q * ED),
                "maskadd": np.ascontiguousarray(maskadd[:, ks]),
                "wh": wh_dev,
                "rhs5": rhs5,
                "lhsT5": lhsT5.astype(NP_BF16),
                "bdt": bdt,
            }
        )
    return in_maps


def kernel(h, mch_mask, comp_val, W, W_edge, a, trace=False):
    h = np.asarray(h, np.float32)
    mch_mask = np.asarray(mch_mask)
    comp_val = np.asarray(comp_val, np.float32)
    W = np.asarray(W, np.float32)
    W_edge = np.asarray(W_edge, np.float32)
    a = np.asarray(a, np.float32)

    in_maps = _host_prep(h, mch_mask, comp_val, W, W_edge, a)

    if "nc" not in _compiled:
        _compiled["nc"] = build_nc()
    nc = _compiled["nc"]

    res = run_bass_kernel_spmd(nc, in_maps, core_ids=list(range(NCORES)), trace=trace)

    out = np.empty((B, M, H * D), np.float32)
    for core in range(NCORES):
        out[:, core * KS : (core + 1) * KS, :] = res.results[core]["out"]
    if trace:
        return out, res
    return out
